# revision 1
# baseline (speedup 1.0000x reference)
"""Multi-head attention with RoPE on 8 Trainium2 NeuronCores.

Sharding: core c -> (batch g = c//4, head-group hg = c%4 of 4 heads).
Per core: QKV projection (column slice of w_qkv), RoPE (rotation via a
signed-permutation matmul + elementwise combine), attention computed as
S^T = K' Q'^T per 128-row j-tile (contraction d=64), exp on ScalarE
(no max-subtraction needed: scores are O(1) by construction), ones-column
appended to V so the softmax denominator falls out of the same PSUM
accumulation as P@V, per-row normalize, then the local out-projection
partial (rows of w_out for the local heads). Partials are summed across
each 4-core batch group with chunked ReduceScatter (one per 512-row
i-block, overlapped with compute); each core lands a distinct
[256-outcol x 512-row] quarter per chunk, and the host just transposes
and concatenates. Matmuls run as float32r (TF32-like) for 4x PE
throughput over fp32.
"""

import numpy as np

H, HD = 16, 64
B, N, DIM = 2, 2048, 1024
N_CORES = 8
GROUPS = [[0, 1, 2, 3], [4, 5, 6, 7]]

_COMPILED = {}


def _host_prep(x, w_qkv, w_out, b_out):
    freqs = 10000.0 ** (-np.arange(0, HD, 2, dtype=np.float32) / HD)
    angles = np.arange(N, dtype=np.float32)[:, None] * freqs
    sin = np.sin(angles).astype(np.float32)
    cos = np.cos(angles).astype(np.float32)
    sin_i = np.stack([sin, sin], axis=-1).reshape(N, HD)
    cos_i = np.stack([cos, cos], axis=-1).reshape(N, HD)
    cs = np.concatenate([cos_i.T, cos_i.T], 0).copy()  # [128, N]
    sn = np.concatenate([sin_i.T, sin_i.T], 0).copy()

    R = np.zeros((HD, HD), np.float32)
    for d in range(32):
        R[d, 2 * d + 1] = -1.0
    for d in range(32, 64):
        R[d, 2 * (d - 32)] = 1.0
    R2 = np.zeros((128, 128), np.float32)
    R2[:64, :64] = R
    R2[64:, 64:] = R
    r2t = np.ascontiguousarray(R2.T)

    in_maps = []
    for c in range(N_CORES):
        g, hg = c // 4, c % 4
        heads = range(4 * hg, 4 * hg + 4)
        w_qk = np.concatenate(
            [np.concatenate([w_qkv[:, h * 64:(h + 1) * 64],
                             w_qkv[:, DIM + h * 64: DIM + (h + 1) * 64]], axis=1)
             for h in heads], axis=1)
        w_v = np.concatenate(
            [w_qkv[:, 2 * DIM + h * 64: 2 * DIM + (h + 1) * 64] for h in heads], axis=1)
        w_o = np.ascontiguousarray(w_out[4 * hg * 64:(4 * hg + 4) * 64, :])
        b_o = np.ascontiguousarray((b_out / 4.0).reshape(8, 128).T)
        in_maps.append({
            "x_t": np.ascontiguousarray(x[g].T),
            "w_qk": np.ascontiguousarray(w_qk),
            "w_v": np.ascontiguousarray(w_v),
            "w_o": w_o,
            "b_o": b_o,
            "cs": cs,
            "sn": sn,
            "r2t": r2t,
            "ones": np.ones((128, 64), np.float32),
        })
    return in_maps


def build_nc(with_collective=True):
    import concourse.bass as bass  # noqa: F401
    import concourse.mybir as mybir
    import concourse.tile as tile
    from concourse import bacc

    f32 = mybir.dt.float32
    f32r = mybir.dt.float32r
    mult = mybir.AluOpType.mult
    add = mybir.AluOpType.add
    Exp = mybir.ActivationFunctionType.Exp

    nc = bacc.Bacc("TRN2", target_bir_lowering=False, debug=False,
                   num_devices=N_CORES)
    x_t = nc.dram_tensor("x_t", [DIM, N], f32r, kind="ExternalInput")
    w_qk = nc.dram_tensor("w_qk", [DIM, 512], f32r, kind="ExternalInput")
    w_v = nc.dram_tensor("w_v", [DIM, 256], f32r, kind="ExternalInput")
    w_o = nc.dram_tensor("w_o", [256, DIM], f32r, kind="ExternalInput")
    b_o = nc.dram_tensor("b_o", [128, 8], f32, kind="ExternalInput")
    cs_d = nc.dram_tensor("cs", [128, N], f32, kind="ExternalInput")
    sn_d = nc.dram_tensor("sn", [128, N], f32, kind="ExternalInput")
    r2t_d = nc.dram_tensor("r2t", [128, 128], f32r, kind="ExternalInput")
    ones_d = nc.dram_tensor("ones", [128, 64], f32r, kind="ExternalInput")
    y_out = nc.dram_tensor("y", [4, 256, 512], f32, kind="ExternalOutput")

    with tile.TileContext(nc) as tc:
        with (
            tc.tile_pool(name="persist", bufs=1) as persist,
            tc.tile_pool(name="ppS", bufs=2, space="PSUM") as ppS,
            tc.tile_pool(name="ppO", bufs=1, space="PSUM") as ppO,
            tc.tile_pool(name="ppC", bufs=2, space="PSUM") as ppC,
            tc.tile_pool(name="dram", bufs=8, space="DRAM") as dram,
        ):
            qp = persist.tile([64, 4, N], f32r)            # q'^T per head [d64, n]
            kp = persist.tile([64, 4, N], f32r)            # k'^T per head [d64, n]
            vsb = persist.tile([128, 16, 4, 65], f32r)     # v + ones col, per j-tile
            wo_sb = persist.tile([128, 2, DIM], f32r)
            b_sb = persist.tile([128, 8], f32)
            ones_sb = persist.tile([1, 64], f32r)

            def attn_jts(ihalf, h, ps_o, jts, epl, pre_jt=None):
                for jt in jts:
                    if pre_jt is not None:
                        pre_jt(jt)
                    ps_s = ppS.tile([128, 1024], f32, name="psA")
                    for half in range(2):
                        nc.tensor.matmul(
                            ps_s[:, half * 512:(half + 1) * 512],
                            lhsT=kp[:, h, jt * 128:(jt + 1) * 128],
                            rhs=qp[:, h,
                                   ihalf * 1024 + half * 512:
                                   ihalf * 1024 + (half + 1) * 512],
                            start=True, stop=True,
                        )
                    e_t = epl.tile([128, 1024], f32r, name="e_t")
                    nc.scalar.activation(e_t[:], ps_s[:], Exp, scale=0.125)
                    for half in range(2):
                        nc.tensor.matmul(
                            ps_o[0:65, half * 512:(half + 1) * 512],
                            lhsT=vsb[:, jt, h, :],
                            rhs=e_t[:, half * 512:(half + 1) * 512],
                            start=(jt == 0), stop=(jt == 15),
                        )

            # ---------------- Phase 1: QKV projection + RoPE ----------------
            with (
                tc.tile_pool(name="xw", bufs=1) as xw,
                tc.tile_pool(name="scr", bufs=4) as scr,
                tc.tile_pool(name="e0pool", bufs=3) as e0pool,
            ):
                cs_sb = xw.tile([128, N], f32)
                sn_sb = xw.tile([128, N], f32)
                r2t_sb = xw.tile([128, 128], f32r)
                wqk = xw.tile([128, 8, 512], f32r)
                wv = xw.tile([128, 8, 256], f32r)
                # compute-critical loads on the sync HWDGE FIFO: qk weights,
                # then the x chunks (issued in the ic4 loop right below).
                # Everything else rides the gpsimd SWDGE queues in parallel.
                xt0 = xw.tile([128, 8, 512], f32r, name="xt", bufs=2)
                for kt in range(8):
                    nc.gpsimd.dma_start(wqk[:, kt, :], w_qk[kt * 128:(kt + 1) * 128, :])
                for kt in range(8):
                    nc.sync.dma_start(
                        xt0[:, kt, :], x_t[kt * 128:(kt + 1) * 128, 0:512])
                nc.sync.dma_start(r2t_sb[:], r2t_d.ap())
                nc.gpsimd.dma_start(cs_sb[:], cs_d.ap())
                nc.gpsimd.dma_start(sn_sb[:], sn_d.ap())
                ones_stage = xw.tile([128, 64], f32r)
                nc.gpsimd.dma_start(ones_stage[:], ones_d[:, :])
                nc.gpsimd.dma_start(ones_sb[:], ones_d[0:1, :])
                nc.scalar.copy(
                    vsb[:, :, :, 64:65],
                    ones_stage[:, :].rearrange("p (a b c) -> p a b c", b=4, c=1))
                for kt in range(8):
                    nc.gpsimd.dma_start(wv[:, kt, :], w_v[kt * 128:(kt + 1) * 128, :])
                nc.gpsimd.dma_start(b_sb[:], b_o.ap())
                for kt in range(2):
                    nc.gpsimd.dma_start(wo_sb[:, kt, :], w_o[kt * 128:(kt + 1) * 128, :])

                for ic4 in range(4):
                    isl = slice(ic4 * 512, (ic4 + 1) * 512)
                    if ic4 == 0:
                        xt = xt0
                    else:
                        xt = xw.tile([128, 8, 512], f32r, name="xt", bufs=2)
                        for kt in range(8):
                            nc.sync.dma_start(xt[:, kt, :], x_t[kt * 128:(kt + 1) * 128, isl])
                    # rope chain for head h-1 emitted after head h's qk matmuls
                    # so the rot matmul never heads the PE queue waiting on its
                    # ACT copy round trip
                    pend = None

                    def rope_chain(h, qks):
                        ps_rot = ppC.tile([128, 512], f32, name="psC")
                        nc.tensor.matmul(ps_rot[:, 0:512], lhsT=r2t_sb[:],
                                         rhs=qks[:], start=True, stop=True)
                        t1 = scr.tile([128, 512], f32, name="t1")
                        nc.vector.tensor_tensor(t1[:], qks[:].bitcast(f32), cs_sb[:, isl], op=mult)
                        t2 = scr.tile([128, 512], f32, name="t2")
                        nc.vector.tensor_tensor(t2[:], ps_rot[:, 0:512], sn_sb[:, isl], op=mult)
                        nc.vector.tensor_tensor(qp[:, h, isl], t1[0:64, :], t2[0:64, :], op=add)
                        nc.vector.tensor_tensor(kp[:, h, isl], t1[64:128, :], t2[64:128, :], op=add)

                    for h in range(4):
                        ps_qk = ppS.tile([128, 1024], f32, name="psA")
                        for kt in range(8):
                            nc.tensor.matmul(
                                ps_qk[:, 0:512],
                                lhsT=wqk[:, kt, h * 128:(h + 1) * 128],
                                rhs=xt[:, kt, :],
                                start=(kt == 0), stop=(kt == 7),
                            )
                        qks = scr.tile([128, 512], f32r, name="qks")
                        nc.scalar.copy(qks[:], ps_qk[:, 0:512])
                        if pend is not None:
                            rope_chain(*pend)
                        pend = (h, qks)
                    rope_chain(*pend)
                    for it2 in range(4):
                        it = ic4 * 4 + it2
                        ps_v = ppC.tile([128, 512], f32, name="psC")
                        for kt in range(8):
                            nc.tensor.matmul(
                                ps_v[:, 0:256],
                                lhsT=xt[:, kt, it2 * 128:(it2 + 1) * 128],
                                rhs=wv[:, kt, :],
                                start=(kt == 0), stop=(kt == 7),
                            )
                        nc.vector.tensor_copy(
                            vsb[:, it, :, 0:64],
                            ps_v[:, 0:256].rearrange("p (h d) -> p h d", d=64),
                        )
                    if ic4 == 1:
                        ps_o0 = ppO.tile([128, 1024], f32, name="psO")
                        attn_jts(0, 0, ps_o0, range(0, 8), e0pool)
                    elif ic4 == 2:
                        attn_jts(0, 0, ps_o0, range(8, 12), e0pool)
                    elif ic4 == 3:
                        attn_jts(0, 0, ps_o0, range(12, 16), e0pool)

            # ---------------- Phase 2+3: attention, out-proj, RS ----------------
            with (
                tc.tile_pool(name="epool", bufs=8) as epool,
                tc.tile_pool(name="opool", bufs=1) as opool,
                tc.tile_pool(name="npool", bufs=4) as npool,
                tc.tile_pool(name="outp", bufs=8) as outp,
            ):
                osb_all = {}

                def attn_norm(ihalf, h, ps_o):
                    # reciprocal computed in halves so the first broadcast
                    # matmul (at the in-order PE queue head) unblocks after
                    # ~0.6us instead of the full-width reciprocal latency
                    osb = osb_all[ihalf]
                    recip = npool.tile([1, 1024], f32r, name="recip")
                    bc_sb = npool.tile([64, 1024], f32, name="bc_sb")
                    for half in range(2):
                        hs = slice(half * 512, (half + 1) * 512)
                        with nc.allow_low_precision(reason="recip feeds f32r matmul"):
                            nc.vector.reciprocal(recip[:, hs], ps_o[64:65, hs])
                        ps_b = ppC.tile([128, 512], f32, name="psC")
                        nc.tensor.matmul(
                            ps_b[0:64, :],
                            lhsT=ones_sb[:],
                            rhs=recip[:, hs],
                            start=True, stop=True,
                        )
                        nc.vector.tensor_copy(bc_sb[:, hs], ps_b[0:64, :])
                    nc.vector.tensor_tensor(
                        osb[h // 2][(h % 2) * 64:(h % 2) * 64 + 64, :],
                        ps_o[0:64, :], bc_sb[:], op=mult)

                def attn_head(ihalf, h):
                    ps_o = ppO.tile([128, 1024], f32, name="psO")
                    attn_jts(ihalf, h, ps_o, range(16), epool)
                    attn_norm(ihalf, h, ps_o)

                def outproj_oc(ihalf, half, oc, rs_in):
                    osb = osb_all[ihalf]
                    ps_out = ppC.tile([128, 512], f32, name="psC")
                    for kt in range(2):
                        nc.tensor.matmul(
                            ps_out[:, :],
                            lhsT=wo_sb[:, kt, oc * 128:(oc + 1) * 128],
                            rhs=osb[kt][:, half * 512:(half + 1) * 512],
                            start=(kt == 0), stop=(kt == 1),
                        )
                    o_t = outp.tile([128, 512], f32, name="o_t")
                    nc.vector.tensor_scalar_add(o_t[:], ps_out[:, :],
                                                b_sb[:, oc:oc + 1])
                    nc.sync.dma_start(rs_in[oc * 128:(oc + 1) * 128, :], o_t[:])

                def rs_fire(ib, rs_in):
                    if with_collective:
                        rs_out = dram.tile([256, 512], f32, name=f"rs_out_{ib}")
                        nc.gpsimd.collective_compute(
                            "ReduceScatter",
                            mybir.AluOpType.add,
                            replica_groups=GROUPS,
                            ins=[rs_in[:]],
                            outs=[rs_out[:]],
                        )
                        nc.sync.dma_start(y_out[ib], rs_out[:])
                    else:
                        nc.sync.dma_start(y_out[ib], rs_in[0:256, :])

                def attn_head_carrying(ihalf, h, co_ihalf, co_half):
                    # run a head's attention with the previous i-half's
                    # out-projection spread one oc per 2 j-tiles, filling the
                    # ACT-bound per-jt PE slack instead of bursting 16 matmuls
                    ib = 2 * co_ihalf + co_half
                    rs_in = dram.tile([1024, 512], f32, name=f"rs_in_{ib}")

                    def pre(jt):
                        if jt % 2 == 1:
                            outproj_oc(co_ihalf, co_half, jt // 2, rs_in)
                    ps_o = ppO.tile([128, 1024], f32, name="psO")
                    attn_jts(ihalf, h, ps_o, range(16), epool, pre)
                    attn_norm(ihalf, h, ps_o)
                    rs_fire(ib, rs_in)

                def outproj_rs(ihalf, half):
                    ib = 2 * ihalf + half
                    rs_in = dram.tile([1024, 512], f32, name=f"rs_in_{ib}")
                    for oc in range(8):
                        outproj_oc(ihalf, half, oc, rs_in)
                    if with_collective:
                        rs_out = dram.tile([256, 512], f32, name=f"rs_out_{ib}")
                        nc.gpsimd.collective_compute(
                            "ReduceScatter",
                            mybir.AluOpType.add,
                            replica_groups=GROUPS,
                            ins=[rs_in[:]],
                            outs=[rs_out[:]],
                        )
                        nc.sync.dma_start(y_out[ib], rs_out[:])
                    else:
                        nc.sync.dma_start(y_out[ib], rs_in[0:256, :])

                # interleave ihalf=0's out-projection between ihalf=1's heads so
                # its PSUM slots and DVE drains overlap ACT-bound attention
                osb_all[0] = [opool.tile([128, 1024], f32r, name=f"osb0_{kt}") for kt in range(2)]
                osb_all[1] = [opool.tile([128, 1024], f32r, name=f"osb1_{kt}") for kt in range(2)]
                attn_norm(0, 0, ps_o0)  # j-loop ran interleaved with phase 1
                for h in range(1, 4):
                    attn_head(0, h)
                attn_head(1, 0)
                attn_head_carrying(1, 1, 0, 0)
                attn_head_carrying(1, 2, 0, 1)
                attn_head(1, 3)
                outproj_rs(1, 0)
                outproj_rs(1, 1)

    nc.compile()
    return nc


def _get_nc():
    if "nc" not in _COMPILED:
        _COMPILED["nc"] = build_nc()
    return _COMPILED["nc"]


def kernel(x, w_qkv, w_out, b_out):
    from concourse import bass_utils

    x = np.asarray(x, dtype=np.float32)
    w_qkv = np.asarray(w_qkv, dtype=np.float32)
    w_out = np.asarray(w_out, dtype=np.float32)
    b_out = np.asarray(b_out, dtype=np.float32)

    nc = _get_nc()
    in_maps = _host_prep(x, w_qkv, w_out, b_out)
    res = bass_utils.run_bass_kernel_spmd(nc, in_maps, list(range(N_CORES)))

    out = np.zeros((B, N, DIM), np.float32)
    for c in range(N_CORES):
        g, pos = c // 4, c % 4
        y = res.results[c]["y"]  # [4, 256, 512]
        for ib in range(4):
            out[g, ib * 512:(ib + 1) * 512, pos * 256:(pos + 1) * 256] = y[ib].T
    return out


if __name__ == "__main__":
    rng = np.random.default_rng(0)
    x = rng.standard_normal((B, N, DIM)).astype(np.float32)
    w_qkv = (rng.standard_normal((DIM, 3 * DIM)) * DIM ** -0.5).astype(np.float32)
    w_out = (rng.standard_normal((DIM, DIM)) * DIM ** -0.5).astype(np.float32)
    b_out = np.zeros(DIM, np.float32)
    out = kernel(x, w_qkv, w_out, b_out)
    print("out", out.shape, out.dtype, float(np.abs(out).max()))



# revision 2
# speedup vs baseline: 1.0013x; 1.0013x over previous
"""Multi-head attention with RoPE on 8 Trainium2 NeuronCores — v2 schedule.

Same math/layout as v1 (core c -> batch g = c//4, head-group c%4; QKV via
column-sliced w_qkv; RoPE as signed-permutation matmul + elementwise; S^T =
K'Q'^T per 128-row j-tile; exp on ACT with no max-subtraction; ones-column
appended to V so the denominator accumulates in the same PSUM as P@V;
chunked ReduceScatter per 512-row i-block). v2 reworks the schedule around
the engine balance (PE ~167us, ACT-exp ~133us, DVE/Pool well under):

- softmax normalize fully off the PE: DVE reciprocal -> GpSimd
  partition_broadcast -> DVE multiply, emitted group-wise so the in-order
  DVE queue never head-blocks on Pool.
- attention inner loop emits with one-jt lookahead (scores jt+1 ahead of
  PV jt) plus dripped PE filler units, so the ACT-bound exp pipeline never
  starves the PE.
- h2/h3's qk projection for the second token half is deferred into the
  early phase-2 blocks as filler; scores get a dedicated PSUM pool so the
  projection/rope PSUM ring never gates them.
- out-projection tail: contributions of heads 0-2 (+bias) are stashed to
  SBUF during the last block; after the final norm only contraction-64
  matmuls for the last head plus a DVE/Pool/ACT-split finalize remain.
"""

import numpy as np
import ml_dtypes

H, HD = 16, 64
B, N, DIM = 2, 2048, 1024
N_CORES = 8
GROUPS = [[0, 1, 2, 3], [4, 5, 6, 7]]

_COMPILED = {}


def _host_prep(x, w_qkv, w_out, b_out):
    freqs = 10000.0 ** (-np.arange(0, HD, 2, dtype=np.float32) / HD)
    angles = np.arange(N, dtype=np.float32)[:, None] * freqs
    sin = np.sin(angles).astype(np.float32)
    cos = np.cos(angles).astype(np.float32)
    sin_i = np.stack([sin, sin], axis=-1).reshape(N, HD)
    cos_i = np.stack([cos, cos], axis=-1).reshape(N, HD)
    cs = np.concatenate([cos_i.T, cos_i.T], 0).copy()  # [128, N]
    sn = np.concatenate([sin_i.T, sin_i.T], 0).copy()

    R = np.zeros((HD, HD), np.float32)
    for d in range(32):
        R[d, 2 * d + 1] = -1.0
    for d in range(32, 64):
        R[d, 2 * (d - 32)] = 1.0
    R2 = np.zeros((128, 128), np.float32)
    R2[:64, :64] = R
    R2[64:, 64:] = R
    r2t = np.ascontiguousarray(R2.T)

    in_maps = []
    for c in range(N_CORES):
        g, hg = c // 4, c % 4
        heads = range(4 * hg, 4 * hg + 4)
        w_qk = np.concatenate(
            [np.concatenate([w_qkv[:, h * 64:(h + 1) * 64],
                             w_qkv[:, DIM + h * 64: DIM + (h + 1) * 64]], axis=1)
             for h in heads], axis=1)
        w_v = np.concatenate(
            [w_qkv[:, 2 * DIM + h * 64: 2 * DIM + (h + 1) * 64] for h in heads], axis=1)
        w_o = np.ascontiguousarray(w_out[4 * hg * 64:(4 * hg + 4) * 64, :])
        b_o = np.ascontiguousarray((b_out / 4.0).reshape(8, 128).T)
        in_maps.append({
            "x_t": np.ascontiguousarray(x[g].T).astype(ml_dtypes.bfloat16),
            "w_qk": np.ascontiguousarray(w_qk).astype(ml_dtypes.bfloat16),
            "w_v": np.ascontiguousarray(w_v).astype(ml_dtypes.bfloat16),
            "w_o": w_o,
            "b_o": b_o,
            "cs": cs,
            "sn": sn,
            "r2t": r2t,
            "ones": np.ones((128, 64), np.float32),
        })
    return in_maps


def build_nc(with_collective=True):
    import concourse.bass as bass  # noqa: F401
    import concourse.mybir as mybir
    import concourse.tile as tile
    from concourse import bacc

    f32 = mybir.dt.float32
    f32r = mybir.dt.float32r
    bf16 = mybir.dt.bfloat16
    mult = mybir.AluOpType.mult
    add = mybir.AluOpType.add
    Exp = mybir.ActivationFunctionType.Exp
    Ident = mybir.ActivationFunctionType.Identity

    nc = bacc.Bacc("TRN2", target_bir_lowering=False, debug=False,
                   num_devices=N_CORES)
    x_t = nc.dram_tensor("x_t", [DIM, N], bf16, kind="ExternalInput")
    w_qk = nc.dram_tensor("w_qk", [DIM, 512], bf16, kind="ExternalInput")
    w_v = nc.dram_tensor("w_v", [DIM, 256], bf16, kind="ExternalInput")
    w_o = nc.dram_tensor("w_o", [256, DIM], f32r, kind="ExternalInput")
    b_o = nc.dram_tensor("b_o", [128, 8], f32, kind="ExternalInput")
    cs_d = nc.dram_tensor("cs", [128, N], f32, kind="ExternalInput")
    sn_d = nc.dram_tensor("sn", [128, N], f32, kind="ExternalInput")
    r2t_d = nc.dram_tensor("r2t", [128, 128], f32r, kind="ExternalInput")
    ones_d = nc.dram_tensor("ones", [128, 64], f32r, kind="ExternalInput")
    y_out = nc.dram_tensor("y", [4, 256, 512], f32, kind="ExternalOutput")
    y2_out = nc.dram_tensor("y2", [2, 256, 512], bf16, kind="ExternalOutput")

    # tail finalize engine per (half, oc); dve/pool units use a stash slot
    TAIL_ENG = {}
    for half in range(2):
        for oc in range(8):
            TAIL_ENG[(half, oc)] = "act" if oc % 2 == 1 else "dve"
    O_SLOT = {u: i for i, u in enumerate(
        u for u in TAIL_ENG if TAIL_ENG[u] != "act")}

    with tile.TileContext(nc) as tc:
        with (
            tc.tile_pool(name="persist", bufs=1) as persist,
            tc.tile_pool(name="xtp", bufs=3) as xtp,
            tc.tile_pool(name="ppS", bufs=2, space="PSUM") as ppS,
            tc.tile_pool(name="ppO", bufs=1, space="PSUM") as ppO,
            tc.tile_pool(name="ppC", bufs=2, space="PSUM") as ppC,
            tc.tile_pool(name="dram", bufs=8, space="DRAM") as dram,
            tc.tile_pool(name="epool", bufs=3) as epool,
        ):
            qp = persist.tile([64, 4, N], f32r)            # q'^T per head [d64, n]
            kp = persist.tile([64, 4, N], f32r)            # k'^T per head [d64, n]
            vsb = persist.tile([128, 16, 4, 65], f32r)     # v + ones col, per j-tile
            wo_sb = persist.tile([128, 2, DIM], f32r)
            b_sb = persist.tile([128, 8], f32)
            wqk = persist.tile([128, 8, 512], bf16)
            cs_hi = persist.tile([128, 1024], f32)
            sn_hi = persist.tile([128, 1024], f32)
            r2t_sb = persist.tile([128, 128], f32r)

            def cs_at(isl):  # cos slice [128, 512] for token slice isl
                return (cs_hi[:, isl.start - 1024:isl.stop - 1024]
                        if isl.start >= 1024 else cs_lo[:, isl])

            def sn_at(isl):
                return (sn_hi[:, isl.start - 1024:isl.stop - 1024]
                        if isl.start >= 1024 else sn_lo[:, isl])

            def qk_mms(h, xt, pool):
                ps_qk = pool.tile([128, 512], f32, name="psC")
                for kt in range(8):
                    nc.tensor.matmul(
                        ps_qk[:, :],
                        lhsT=wqk[:, kt, h * 128:(h + 1) * 128],
                        rhs=xt[:, kt, :],
                        start=(kt == 0), stop=(kt == 7),
                    )
                return ps_qk

            def qk_copy(h, ps_qk, scrp):
                # alternate ACT/DVE so ring slots free independently
                qks = scrp.tile([128, 512], f32r, name="qks")
                if h % 2 == 0:
                    nc.scalar.copy(qks[:], ps_qk[:, :])
                else:
                    nc.vector.tensor_copy(qks[:], ps_qk[:, :])
                return qks

            # rope rotation + t1/t2; qp/kp adds returned as deferred closure
            def rope_rot(h, isl, qks, scrp, pool):
                ps_rot = pool.tile([128, 512], f32, name="psC")
                nc.tensor.matmul(ps_rot[:, :], lhsT=r2t_sb[:],
                                 rhs=qks[:], start=True, stop=True)
                t1 = scrp.tile([128, 512], f32, name="t1")
                nc.gpsimd.tensor_tensor(t1[:], qks[:].bitcast(f32), cs_at(isl), op=mult)
                t2 = scrp.tile([128, 512], f32, name="t2")
                nc.vector.tensor_tensor(t2[:], ps_rot[:, :], sn_at(isl), op=mult)

                def adds():
                    nc.gpsimd.tensor_tensor(kp[:, h, isl], t1[64:128, :], t2[64:128, :], op=add)
                    nc.vector.tensor_tensor(qp[:, h, isl], t1[0:64, :], t2[0:64, :], op=add)
                return adds

            # ---- attention emitter with one-jt lookahead + PE fillers ----
            def attn_seq(ihalf, h, ps_o, jts, fillers):
                pend_pv = [None]

                def emit_pv(jt, e_t):
                    for half in range(2):
                        nc.tensor.matmul(
                            ps_o[0:65, half * 512:(half + 1) * 512],
                            lhsT=vsb[:, jt, h, :],
                            rhs=e_t[:, half * 512:(half + 1) * 512],
                            start=(jt == 0), stop=(jt == 15),
                        )

                for jt in jts:
                    ps_s = ppS.tile([128, 1024], f32, name="psA")
                    for half in range(2):
                        nc.tensor.matmul(
                            ps_s[:, half * 512:(half + 1) * 512],
                            lhsT=kp[:, h, jt * 128:(jt + 1) * 128],
                            rhs=qp[:, h,
                                   ihalf * 1024 + half * 512:
                                   ihalf * 1024 + (half + 1) * 512],
                            start=True, stop=True,
                        )
                    e_t = epool.tile([128, 1024], f32r, name="e_t")
                    nc.scalar.activation(e_t[:], ps_s[:], Exp, scale=0.125)
                    if pend_pv[0] is not None:
                        if fillers:
                            fillers.pop(0)()
                        emit_pv(*pend_pv[0])
                    pend_pv[0] = (jt, e_t)

                def finish():
                    emit_pv(*pend_pv[0])
                return finish

            # ---------------- Phase 1 ----------------
            with (
                tc.tile_pool(name="xw", bufs=1) as xw,
                tc.tile_pool(name="scr", bufs=3) as scr,
            ):
                cs_lo = xw.tile([128, 1024], f32)
                sn_lo = xw.tile([128, 1024], f32)
                wv = xw.tile([128, 8, 256], bf16)
                xt0 = xtp.tile([128, 8, 512], bf16, name="xt")
                # keep the gpsimd/SWDGE queue nearly empty in phase 1: its
                # descriptor processing occupies the Pool ENGINE (~1us per
                # transfer), which phase 1 needs for rope t1/kp-adds.
                for kt in range(4):
                    nc.sync.dma_start(wqk[:, kt, :], w_qk[kt * 128:(kt + 1) * 128, :])
                    nc.sync.dma_start(xt0[:, kt, :], x_t[kt * 128:(kt + 1) * 128, 0:512])
                for kt in range(4, 8):
                    nc.gpsimd.dma_start(wqk[:, kt, :], w_qk[kt * 128:(kt + 1) * 128, :])
                for kt in range(4, 8):
                    nc.sync.dma_start(xt0[:, kt, :], x_t[kt * 128:(kt + 1) * 128, 0:512])
                nc.sync.dma_start(r2t_sb[:], r2t_d.ap())
                nc.sync.dma_start(cs_lo[:], cs_d[:, 0:1024])
                nc.sync.dma_start(sn_lo[:], sn_d[:, 0:1024])
                for kt in range(8):
                    nc.sync.dma_start(wv[:, kt, :], w_v[kt * 128:(kt + 1) * 128, :])
                ones_stage = xw.tile([128, 64], f32r)
                nc.gpsimd.dma_start(ones_stage[:], ones_d[:, :])
                nc.scalar.copy(
                    vsb[:, :, :, 64:65],
                    ones_stage[:, :].rearrange("p (a b c) -> p a b c", b=4, c=1))
                nc.sync.dma_start(cs_hi[:], cs_d[:, 1024:2048])
                nc.sync.dma_start(sn_hi[:], sn_d[:, 1024:2048])
                nc.gpsimd.dma_start(b_sb[:], b_o.ap())
                for kt in range(2):
                    nc.sync.dma_start(wo_sb[:, kt, :], w_o[kt * 128:(kt + 1) * 128, :])

                xts = {0: xt0}
                ph1_fin = [None]
                JTS_TOP = {2: range(4, 8), 3: range(10, 12)}
                JTS_END = {1: range(0, 4), 2: range(8, 10), 3: range(12, 16)}
                for ic4 in range(4):
                    isl = slice(ic4 * 512, (ic4 + 1) * 512)
                    if ic4 < 3:
                        nsl = slice((ic4 + 1) * 512, (ic4 + 2) * 512)
                        xn = xtp.tile([128, 8, 512], bf16, name="xt")
                        xts[ic4 + 1] = xn
                        for kt in range(8):
                            nc.sync.dma_start(xn[:, kt, :], x_t[kt * 128:(kt + 1) * 128, nsl])
                    xt = xts[ic4]
                    if ic4 in JTS_TOP:
                        ph1_fin[0]()
                        ph1_fin[0] = attn_seq(0, 0, ps_o0, JTS_TOP[ic4], [])
                    heads = list(range(4) if ic4 < 2 else range(2))
                    defer = 2 if len(heads) == 4 else 1
                    qks_of = {}
                    adds = []
                    n_rot = 0

                    def emit_rot(hh):
                        adds.append(rope_rot(hh, isl, qks_of[hh], scr, ppC))
                        if len(adds) > 1:
                            adds.pop(0)()

                    for idx, h in enumerate(heads):
                        ps_qk = qk_mms(h, xt, ppC)
                        qks_of[h] = qk_copy(h, ps_qk, scr)
                        while idx - n_rot >= defer:
                            emit_rot(heads[n_rot])
                            n_rot += 1
                    while n_rot < len(heads):
                        emit_rot(heads[n_rot])
                        n_rot += 1
                    for it2 in range(4):
                        it = ic4 * 4 + it2
                        ps_v = ppC.tile([128, 512], f32, name="psC")
                        for kt in range(8):
                            nc.tensor.matmul(
                                ps_v[:, 0:256],
                                lhsT=xt[:, kt, it2 * 128:(it2 + 1) * 128],
                                rhs=wv[:, kt, :],
                                start=(kt == 0), stop=(kt == 7),
                            )
                        nc.vector.tensor_copy(
                            vsb[:, it, :, 0:64],
                            ps_v[:, 0:256].rearrange("p (h d) -> p h d", d=64),
                        )
                    while adds:
                        adds.pop(0)()
                    if ic4 == 1:
                        ps_o0 = ppO.tile([128, 1024], f32, name="psO")
                    if ic4 >= 1:
                        if ph1_fin[0] is not None:
                            ph1_fin[0]()
                        ph1_fin[0] = attn_seq(0, 0, ps_o0, JTS_END[ic4], [])

            # ---------------- Phase 2 ----------------
            with (
                tc.tile_pool(name="opool", bufs=1) as opool,
                tc.tile_pool(name="npool", bufs=1) as npool,
                tc.tile_pool(name="outp", bufs=3) as outp,
                tc.tile_pool(name="toutp", bufs=8) as toutp,
                tc.tile_pool(name="pscr", bufs=1) as pscr,
            ):
                osb_all = {
                    0: [opool.tile([128, 1024], f32r, name=f"osb0_{kt}") for kt in range(2)],
                    1: [opool.tile([128, 1024], f32r, name=f"osb1_{kt}") for kt in range(2)],
                }
                # heads 0-2 + bias partials of the last i-half's out-proj
                o_part = opool.tile([128, len(O_SLOT), 512], bf16)
                rs_ins = {ib: dram.tile([1024, 512], f32 if ib < 2 else bf16,
                                         name=f"rs_in_{ib}")
                          for ib in range(4)}

                def attn_norm(ihalf, h, ps_o, nsl=2):
                    # grouped emission: recips, then broadcasts, then mults
                    osb = osb_all[ihalf]
                    recip = npool.tile([1, 1024], f32r, name="recip")
                    bc_sb = npool.tile([64, 1024], f32r, name="bc_sb")
                    w = 1024 // nsl
                    halves = [slice(i * w, (i + 1) * w) for i in range(nsl)]
                    for hs in halves:
                        with nc.allow_low_precision(reason="softmax denom recip"):
                            nc.vector.reciprocal(recip[:, hs], ps_o[64:65, hs])
                    for hs in halves:
                        nc.gpsimd.partition_broadcast(bc_sb[:, hs], recip[0:1, hs])
                    for hs in halves:
                        nc.vector.tensor_tensor(
                            osb[h // 2][(h % 2) * 64:(h % 2) * 64 + 64, hs],
                            ps_o[0:64, hs], bc_sb[:, hs], op=mult)

                # --- filler units ---
                def outproj_full(ihalf, half, oc):
                    osb = osb_all[ihalf]
                    ps_out = ppC.tile([128, 512], f32, name="psC")
                    for kt in range(2):
                        nc.tensor.matmul(
                            ps_out[:, :],
                            lhsT=wo_sb[:, kt, oc * 128:(oc + 1) * 128],
                            rhs=osb[kt][:, half * 512:(half + 1) * 512],
                            start=(kt == 0), stop=(kt == 1),
                        )
                    o_t = outp.tile([128, 512], f32, name="o_t")
                    nc.vector.tensor_scalar_add(o_t[:], ps_out[:, :],
                                                b_sb[:, oc:oc + 1])
                    ib = 2 * ihalf + half
                    nc.sync.dma_start(rs_ins[ib][oc * 128:(oc + 1) * 128, :], o_t[:])

                def outproj_stash(half, oc):
                    # heads 0,1 (kt0) + head 2 (kt1 lower 64) + bias -> SBUF
                    ps_out = ppC.tile([128, 512], f32, name="psC")
                    nc.tensor.matmul(
                        ps_out[:, :],
                        lhsT=wo_sb[:, 0, oc * 128:(oc + 1) * 128],
                        rhs=osb_all[1][0][:, half * 512:(half + 1) * 512],
                        start=True, stop=False,
                    )
                    nc.tensor.matmul(
                        ps_out[:, :],
                        lhsT=wo_sb[0:64, 1, oc * 128:(oc + 1) * 128],
                        rhs=osb_all[1][1][0:64, half * 512:(half + 1) * 512],
                        start=False, stop=True,
                    )
                    with nc.allow_low_precision(reason="outproj partial stash"):
                        nc.vector.tensor_scalar_add(
                            o_part[:, O_SLOT[(half, oc)], :], ps_out[:, :],
                            b_sb[:, oc:oc + 1])

                def outproj_tail(half, oc):
                    eng = TAIL_ENG[(half, oc)]
                    if oc % 2 == 0:
                        ps_out = ppC.tile([128, 512], f32, name="psC")
                    else:
                        ps_out = ppS.tile([128, 512], f32, name="psA")
                    o_t = toutp.tile([128, 512], bf16, name="o_t2")
                    if eng == "act":
                        for kt in range(2):
                            nc.tensor.matmul(
                                ps_out[:, :],
                                lhsT=wo_sb[:, kt, oc * 128:(oc + 1) * 128],
                                rhs=osb_all[1][kt][:, half * 512:(half + 1) * 512],
                                start=(kt == 0), stop=(kt == 1),
                            )
                        with nc.allow_low_precision(reason="bf16 tail chunk"):
                            nc.scalar.activation(o_t[:], ps_out[:, :], Ident,
                                                 bias=b_sb[:, oc:oc + 1])
                    else:
                        nc.tensor.matmul(
                            ps_out[:, :],
                            lhsT=wo_sb[64:128, 1, oc * 128:(oc + 1) * 128],
                            rhs=osb_all[1][1][64:128, half * 512:(half + 1) * 512],
                            start=True, stop=True,
                        )
                        with nc.allow_low_precision(reason="bf16 tail chunk"):
                            nc.vector.tensor_tensor(
                                o_t[:], ps_out[:, :],
                                o_part[:, O_SLOT[(half, oc)], :], op=add)
                    dq = nc.sync if oc % 4 < 2 else nc.gpsimd
                    dq.dma_start(rs_ins[2 + half][oc * 128:(oc + 1) * 128, :], o_t[:])

                def deferred_qk_units(h, ic4):
                    isl = slice(ic4 * 512, (ic4 + 1) * 512)
                    xt = xts[ic4]
                    st = {}

                    def mm_pair(i):
                        def f():
                            if i == 0:
                                st["ps"] = ppC.tile([128, 512], f32, name="psC")
                            for kt in (2 * i, 2 * i + 1):
                                nc.tensor.matmul(
                                    st["ps"][:, :],
                                    lhsT=wqk[:, kt, h * 128:(h + 1) * 128],
                                    rhs=xt[:, kt, :],
                                    start=(kt == 0), stop=(kt == 7),
                                )
                        return f

                    def rope_unit():
                        qks = pscr.tile([128, 512], f32r, name="qks")
                        nc.vector.tensor_copy(qks[:], st["ps"][:, :])
                        rope_rot(h, isl, qks, pscr, ppC)()

                    return [mm_pair(i) for i in range(4)] + [rope_unit]

                def run_block(ihalf, h, fillers, nsl=2):
                    ps_o = ppO.tile([128, 1024], f32, name="psO")
                    fin = attn_seq(ihalf, h, ps_o, range(16), fillers)
                    while fillers:
                        fillers.pop(0)()
                    fin()
                    attn_norm(ihalf, h, ps_o, nsl)

                def rs_fire(ib):
                    dt = f32 if ib < 2 else bf16
                    dst = y_out[ib] if ib < 2 else y2_out[ib - 2]
                    if with_collective:
                        rs_out = dram.tile([256, 512], dt, name=f"rs_out_{ib}")
                        nc.gpsimd.collective_compute(
                            "ReduceScatter",
                            mybir.AluOpType.add,
                            replica_groups=GROUPS,
                            ins=[rs_ins[ib][:]],
                            outs=[rs_out[:]],
                        )
                        nc.sync.dma_start(dst, rs_out[:])
                    else:
                        nc.sync.dma_start(dst, rs_ins[ib][0:256, :])

                # finish interleaved block (0,0)
                ph1_fin[0]()
                attn_norm(0, 0, ps_o0)

                run_block(0, 1, deferred_qk_units(2, 2) + deferred_qk_units(2, 3))
                run_block(0, 2, deferred_qk_units(3, 2))
                run_block(0, 3, deferred_qk_units(3, 3))
                run_block(1, 0, [lambda oc=oc: outproj_full(0, 0, oc) for oc in range(8)])
                rs_fire(0)
                run_block(1, 1, [lambda oc=oc: outproj_full(0, 1, oc) for oc in range(5)])
                run_block(1, 2, [lambda oc=oc: outproj_full(0, 1, oc) for oc in range(5, 8)])
                rs_fire(1)
                run_block(1, 3, [lambda u=u: outproj_stash(*u)
                                 for u in sorted(O_SLOT, key=O_SLOT.get)], nsl=4)
                # tail: only head-3 matmuls (dve/pool units) or full 2-kt
                # (act units); ocs 0,1 first so the y copy can start early
                for half in range(2):
                    for oc in range(8):
                        outproj_tail(half, oc)
                    rs_fire(2 + half)

    nc.compile()
    return nc


def _get_nc():
    if "nc" not in _COMPILED:
        _COMPILED["nc"] = build_nc()
    return _COMPILED["nc"]


def kernel(x, w_qkv, w_out, b_out):
    from concourse import bass_utils

    x = np.asarray(x, dtype=np.float32)
    w_qkv = np.asarray(w_qkv, dtype=np.float32)
    w_out = np.asarray(w_out, dtype=np.float32)
    b_out = np.asarray(b_out, dtype=np.float32)

    nc = _get_nc()
    in_maps = _host_prep(x, w_qkv, w_out, b_out)
    res = bass_utils.run_bass_kernel_spmd(nc, in_maps, list(range(N_CORES)))

    out = np.zeros((B, N, DIM), np.float32)
    for c in range(N_CORES):
        g, pos = c // 4, c % 4
        y = res.results[c]["y"]  # [4, 256, 512] (ib 0,1 valid)
        y2 = np.asarray(res.results[c]["y2"]).astype(np.float32)
        for ib in range(4):
            blk = y[ib] if ib < 2 else y2[ib - 2]
            out[g, ib * 512:(ib + 1) * 512, pos * 256:(pos + 1) * 256] = blk.T
    return out


if __name__ == "__main__":
    rng = np.random.default_rng(0)
    x = rng.standard_normal((B, N, DIM)).astype(np.float32)
    w_qkv = (rng.standard_normal((DIM, 3 * DIM)) * DIM ** -0.5).astype(np.float32)
    w_out = (rng.standard_normal((DIM, DIM)) * DIM ** -0.5).astype(np.float32)
    b_out = np.zeros(DIM, np.float32)
    out = kernel(x, w_qkv, w_out, b_out)
    print("out", out.shape, out.dtype, float(np.abs(out).max()))


# revision 3
# speedup vs baseline: 1.0189x; 1.0176x over previous
"""Multi-head attention with RoPE on 8 Trainium2 NeuronCores — v2 schedule.

Same math/layout as v1 (core c -> batch g = c//4, head-group c%4; QKV via
column-sliced w_qkv; RoPE as signed-permutation matmul + elementwise; S^T =
K'Q'^T per 128-row j-tile; exp on ACT with no max-subtraction; ones-column
appended to V so the denominator accumulates in the same PSUM as P@V;
chunked ReduceScatter per 512-row i-block). v2 reworks the schedule around
the engine balance (PE ~167us, ACT-exp ~133us, DVE/Pool well under):

- softmax normalize fully off the PE: DVE reciprocal -> GpSimd
  partition_broadcast -> DVE multiply, emitted group-wise so the in-order
  DVE queue never head-blocks on Pool.
- attention inner loop emits with one-jt lookahead (scores jt+1 ahead of
  PV jt) plus dripped PE filler units, so the ACT-bound exp pipeline never
  starves the PE.
- h2/h3's qk projection for the second token half is deferred into the
  early phase-2 blocks as filler; scores get a dedicated PSUM pool so the
  projection/rope PSUM ring never gates them.
- out-projection tail: contributions of heads 0-2 (+bias) are stashed to
  SBUF during the last block; after the final norm only contraction-64
  matmuls for the last head plus a DVE/Pool/ACT-split finalize remain.
"""

import numpy as np
import ml_dtypes

H, HD = 16, 64
B, N, DIM = 2, 2048, 1024
N_CORES = 8
GROUPS = [[0, 1, 2, 3], [4, 5, 6, 7]]

_COMPILED = {}


def _host_prep(x, w_qkv, w_out, b_out):
    freqs = 10000.0 ** (-np.arange(0, HD, 2, dtype=np.float32) / HD)
    angles = np.arange(N, dtype=np.float32)[:, None] * freqs
    sin = np.sin(angles).astype(np.float32)
    cos = np.cos(angles).astype(np.float32)
    sin_i = np.stack([sin, sin], axis=-1).reshape(N, HD)
    cos_i = np.stack([cos, cos], axis=-1).reshape(N, HD)
    cs = np.concatenate([cos_i.T, cos_i.T], 0).copy()  # [128, N]
    sn = np.concatenate([sin_i.T, sin_i.T], 0).copy()

    R = np.zeros((HD, HD), np.float32)
    for d in range(32):
        R[d, 2 * d + 1] = -1.0
    for d in range(32, 64):
        R[d, 2 * (d - 32)] = 1.0
    R2 = np.zeros((128, 128), np.float32)
    R2[:64, :64] = R
    R2[64:, 64:] = R
    r2t = np.ascontiguousarray(R2.T)

    in_maps = []
    for c in range(N_CORES):
        g, hg = c // 4, c % 4
        heads = range(4 * hg, 4 * hg + 4)
        w_qk = np.concatenate(
            [np.concatenate([w_qkv[:, h * 64:(h + 1) * 64],
                             w_qkv[:, DIM + h * 64: DIM + (h + 1) * 64]], axis=1)
             for h in heads], axis=1)
        w_v = np.concatenate(
            [w_qkv[:, 2 * DIM + h * 64: 2 * DIM + (h + 1) * 64] for h in heads], axis=1)
        w_o = np.ascontiguousarray(w_out[4 * hg * 64:(4 * hg + 4) * 64, :])
        b_o = np.ascontiguousarray((b_out / 4.0).reshape(8, 128).T)
        in_maps.append({
            "x_t": np.ascontiguousarray(x[g].T).astype(ml_dtypes.bfloat16),
            "w_qk": np.ascontiguousarray(w_qk).astype(ml_dtypes.bfloat16),
            "w_v": np.ascontiguousarray(w_v).astype(ml_dtypes.bfloat16),
            "w_o": w_o,
            "b_o": b_o,
            "cs": cs,
            "sn": sn,
            "r2t": r2t,
            "ones": np.ones((128, 64), np.float32),
        })
    return in_maps


def build_nc(with_collective=True):
    import concourse.bass as bass  # noqa: F401
    import concourse.mybir as mybir
    import concourse.tile as tile
    from concourse import bacc

    f32 = mybir.dt.float32
    f32r = mybir.dt.float32r
    bf16 = mybir.dt.bfloat16
    mult = mybir.AluOpType.mult
    add = mybir.AluOpType.add
    Exp = mybir.ActivationFunctionType.Exp
    Ident = mybir.ActivationFunctionType.Identity

    nc = bacc.Bacc("TRN2", target_bir_lowering=False, debug=False,
                   num_devices=N_CORES)
    x_t = nc.dram_tensor("x_t", [DIM, N], bf16, kind="ExternalInput")
    w_qk = nc.dram_tensor("w_qk", [DIM, 512], bf16, kind="ExternalInput")
    w_v = nc.dram_tensor("w_v", [DIM, 256], bf16, kind="ExternalInput")
    w_o = nc.dram_tensor("w_o", [256, DIM], f32r, kind="ExternalInput")
    b_o = nc.dram_tensor("b_o", [128, 8], f32, kind="ExternalInput")
    cs_d = nc.dram_tensor("cs", [128, N], f32, kind="ExternalInput")
    sn_d = nc.dram_tensor("sn", [128, N], f32, kind="ExternalInput")
    r2t_d = nc.dram_tensor("r2t", [128, 128], f32r, kind="ExternalInput")
    ones_d = nc.dram_tensor("ones", [128, 64], f32r, kind="ExternalInput")
    y_out = nc.dram_tensor("y", [4, 256, 512], f32, kind="ExternalOutput")
    y2_out = nc.dram_tensor("y2", [2, 256, 512], bf16, kind="ExternalOutput")

    # tail finalize engine per (half, oc); dve/pool units use a stash slot
    TAIL_ENG = {}
    for half in range(2):
        for oc in range(8):
            TAIL_ENG[(half, oc)] = "act" if oc % 2 == 1 else "dve"
    O_SLOT = {u: i for i, u in enumerate(
        u for u in TAIL_ENG if TAIL_ENG[u] != "act")}

    with tile.TileContext(nc) as tc:
        with (
            tc.tile_pool(name="persist", bufs=1) as persist,
            tc.tile_pool(name="xtp", bufs=4) as xtp,
            tc.tile_pool(name="ppS", bufs=2, space="PSUM") as ppS,
            tc.tile_pool(name="ppO", bufs=1, space="PSUM") as ppO,
            tc.tile_pool(name="ppC", bufs=2, space="PSUM") as ppC,
            tc.tile_pool(name="dram", bufs=8, space="DRAM") as dram,
            tc.tile_pool(name="epool", bufs=3) as epool,
        ):
            qp = persist.tile([64, 4, N], f32r)            # q'^T per head [d64, n]
            kp = persist.tile([64, 4, N], f32r)            # k'^T per head [d64, n]
            vsb = persist.tile([128, 16, 4, 65], bf16)     # v + ones col, per j-tile
            wo_sb = persist.tile([128, 2, DIM], f32r)
            b_sb = persist.tile([128, 8], f32)
            wqk = persist.tile([128, 8, 512], bf16)
            cs_hi = persist.tile([128, 1024], f32)
            sn_hi = persist.tile([128, 1024], f32)
            r2t_sb = persist.tile([128, 128], f32r)

            def cs_at(isl):  # cos slice [128, 512] for token slice isl
                return (cs_hi[:, isl.start - 1024:isl.stop - 1024]
                        if isl.start >= 1024 else cs_lo[:, isl])

            def sn_at(isl):
                return (sn_hi[:, isl.start - 1024:isl.stop - 1024]
                        if isl.start >= 1024 else sn_lo[:, isl])

            def qk_mms(h, xt, pool):
                ps_qk = pool.tile([128, 512], f32, name="psC")
                for kt in range(8):
                    nc.tensor.matmul(
                        ps_qk[:, :],
                        lhsT=wqk[:, kt, h * 128:(h + 1) * 128],
                        rhs=xt[:, kt, :],
                        start=(kt == 0), stop=(kt == 7),
                    )
                return ps_qk

            def qk_copy(h, ps_qk, scrp):
                # alternate ACT/DVE so ring slots free independently
                qks = scrp.tile([128, 512], f32r, name="qks")
                if h % 2 == 0:
                    nc.scalar.copy(qks[:], ps_qk[:, :])
                else:
                    nc.vector.tensor_copy(qks[:], ps_qk[:, :])
                return qks

            # rope rotation + t1/t2; qp/kp adds returned as deferred closure
            def rope_rot(h, isl, qks, scrp, pool):
                ps_rot = pool.tile([128, 512], f32, name="psC")
                nc.tensor.matmul(ps_rot[:, :], lhsT=r2t_sb[:],
                                 rhs=qks[:], start=True, stop=True)
                t1 = scrp.tile([128, 512], f32, name="t1")
                nc.gpsimd.tensor_tensor(t1[:], qks[:].bitcast(f32), cs_at(isl), op=mult)
                t2 = scrp.tile([128, 512], f32, name="t2")
                nc.vector.tensor_tensor(t2[:], ps_rot[:, :], sn_at(isl), op=mult)

                def adds():
                    nc.gpsimd.tensor_tensor(kp[:, h, isl], t1[64:128, :], t2[64:128, :], op=add)
                    nc.vector.tensor_tensor(qp[:, h, isl], t1[0:64, :], t2[0:64, :], op=add)
                return adds

            # ---- attention emitter with one-jt lookahead + PE fillers ----
            def attn_seq(ihalf, h, ps_o, jts, fillers):
                pend_pv = [None]

                def emit_pv(jt, e_t):
                    for half in range(2):
                        nc.tensor.matmul(
                            ps_o[0:65, half * 512:(half + 1) * 512],
                            lhsT=vsb[:, jt, h, :],
                            rhs=e_t[:, half * 512:(half + 1) * 512],
                            start=(jt == 0), stop=(jt == 15),
                        )

                for jt in jts:
                    ps_s = ppS.tile([128, 1024], f32, name="psA")
                    for half in range(2):
                        nc.tensor.matmul(
                            ps_s[:, half * 512:(half + 1) * 512],
                            lhsT=kp[:, h, jt * 128:(jt + 1) * 128],
                            rhs=qp[:, h,
                                   ihalf * 1024 + half * 512:
                                   ihalf * 1024 + (half + 1) * 512],
                            start=True, stop=True,
                        )
                    e_t = epool.tile([128, 1024], bf16, name="e_t")
                    nc.scalar.activation(e_t[:], ps_s[:], Exp, scale=0.125)
                    if pend_pv[0] is not None:
                        if fillers:
                            fillers.pop(0)()
                        emit_pv(*pend_pv[0])
                    pend_pv[0] = (jt, e_t)

                def finish():
                    emit_pv(*pend_pv[0])
                return finish

            # ---------------- Phase 1 ----------------
            with (
                tc.tile_pool(name="xw", bufs=1) as xw,
                tc.tile_pool(name="scr", bufs=3) as scr,
            ):
                cs_lo = xw.tile([128, 1024], f32)
                sn_lo = xw.tile([128, 1024], f32)
                wv = xw.tile([128, 8, 256], bf16)
                xt0 = xtp.tile([128, 8, 512], bf16, name="xt")
                # keep the gpsimd/SWDGE queue nearly empty in phase 1: its
                # descriptor processing occupies the Pool ENGINE (~1us per
                # transfer), which phase 1 needs for rope t1/kp-adds.
                for kt in range(4):
                    nc.sync.dma_start(wqk[:, kt, :], w_qk[kt * 128:(kt + 1) * 128, :])
                    nc.sync.dma_start(xt0[:, kt, :], x_t[kt * 128:(kt + 1) * 128, 0:512])
                for kt in range(4, 8):
                    nc.gpsimd.dma_start(wqk[:, kt, :], w_qk[kt * 128:(kt + 1) * 128, :])
                for kt in range(4, 8):
                    nc.sync.dma_start(xt0[:, kt, :], x_t[kt * 128:(kt + 1) * 128, 0:512])
                nc.sync.dma_start(r2t_sb[:], r2t_d.ap())
                nc.sync.dma_start(cs_lo[:], cs_d[:, 0:1024])
                nc.sync.dma_start(sn_lo[:], sn_d[:, 0:1024])
                for kt in range(8):
                    nc.sync.dma_start(wv[:, kt, :], w_v[kt * 128:(kt + 1) * 128, :])
                ones_stage = xw.tile([128, 64], f32r)
                nc.gpsimd.dma_start(ones_stage[:], ones_d[:, :])
                nc.scalar.copy(
                    vsb[:, :, :, 64:65],
                    ones_stage[:, :].rearrange("p (a b c) -> p a b c", b=4, c=1))
                nc.sync.dma_start(cs_hi[:], cs_d[:, 1024:2048])
                nc.sync.dma_start(sn_hi[:], sn_d[:, 1024:2048])
                nc.gpsimd.dma_start(b_sb[:], b_o.ap())
                for kt in range(2):
                    nc.sync.dma_start(wo_sb[:, kt, :], w_o[kt * 128:(kt + 1) * 128, :])

                xts = {0: xt0}
                ph1_fin = [None]
                JTS_TOP = {2: range(4, 8), 3: range(10, 12)}
                JTS_END = {1: range(0, 4), 2: range(8, 10), 3: range(12, 16)}
                for ic4 in range(4):
                    isl = slice(ic4 * 512, (ic4 + 1) * 512)
                    if ic4 < 3:
                        nsl = slice((ic4 + 1) * 512, (ic4 + 2) * 512)
                        xn = xtp.tile([128, 8, 512], bf16, name="xt")
                        xts[ic4 + 1] = xn
                        for kt in range(8):
                            nc.sync.dma_start(xn[:, kt, :], x_t[kt * 128:(kt + 1) * 128, nsl])
                    xt = xts[ic4]
                    if ic4 in JTS_TOP:
                        ph1_fin[0]()
                        ph1_fin[0] = attn_seq(0, 0, ps_o0, JTS_TOP[ic4], [])
                    heads = list(range(4) if ic4 < 2 else range(2))
                    defer = 2 if len(heads) == 4 else 1
                    qks_of = {}
                    adds = []
                    n_rot = 0

                    def emit_rot(hh):
                        adds.append(rope_rot(hh, isl, qks_of[hh], scr, ppC))
                        if len(adds) > 1:
                            adds.pop(0)()

                    for idx, h in enumerate(heads):
                        ps_qk = qk_mms(h, xt, ppC)
                        qks_of[h] = qk_copy(h, ps_qk, scr)
                        while idx - n_rot >= defer:
                            emit_rot(heads[n_rot])
                            n_rot += 1
                    while n_rot < len(heads):
                        emit_rot(heads[n_rot])
                        n_rot += 1
                    for it2 in range(4):
                        it = ic4 * 4 + it2
                        ps_v = ppC.tile([128, 512], f32, name="psC")
                        for kt in range(8):
                            nc.tensor.matmul(
                                ps_v[:, 0:256],
                                lhsT=xt[:, kt, it2 * 128:(it2 + 1) * 128],
                                rhs=wv[:, kt, :],
                                start=(kt == 0), stop=(kt == 7),
                            )
                        nc.vector.tensor_copy(
                            vsb[:, it, :, 0:64],
                            ps_v[:, 0:256].rearrange("p (h d) -> p h d", d=64),
                        )
                    while adds:
                        adds.pop(0)()
                    if ic4 == 1:
                        ps_o0 = ppO.tile([128, 1024], f32, name="psO")
                    if ic4 >= 1:
                        if ph1_fin[0] is not None:
                            ph1_fin[0]()
                        ph1_fin[0] = attn_seq(0, 0, ps_o0, JTS_END[ic4], [])

            # ---------------- Phase 2 ----------------
            with (
                tc.tile_pool(name="opool", bufs=1) as opool,
                tc.tile_pool(name="npool", bufs=1) as npool,
                tc.tile_pool(name="outp", bufs=3) as outp,
                tc.tile_pool(name="toutp", bufs=8) as toutp,
                tc.tile_pool(name="pscr", bufs=1) as pscr,
            ):
                osb_all = {
                    0: [opool.tile([128, 1024], f32r, name=f"osb0_{kt}") for kt in range(2)],
                    1: [opool.tile([128, 1024], f32r, name=f"osb1_{kt}") for kt in range(2)],
                }
                # heads 0-2 + bias partials of the last i-half's out-proj
                o_part = opool.tile([128, len(O_SLOT), 512], bf16)
                rs_ins = {ib: dram.tile([1024, 512], f32 if ib < 2 else bf16,
                                         name=f"rs_in_{ib}")
                          for ib in range(4)}

                def attn_norm(ihalf, h, ps_o, nsl=2):
                    # grouped emission: recips, then broadcasts, then mults
                    osb = osb_all[ihalf]
                    recip = npool.tile([1, 1024], f32r, name="recip")
                    bc_sb = npool.tile([64, 1024], f32r, name="bc_sb")
                    w = 1024 // nsl
                    halves = [slice(i * w, (i + 1) * w) for i in range(nsl)]
                    for hs in halves:
                        with nc.allow_low_precision(reason="softmax denom recip"):
                            nc.vector.reciprocal(recip[:, hs], ps_o[64:65, hs])
                    for hs in halves:
                        nc.gpsimd.partition_broadcast(bc_sb[:, hs], recip[0:1, hs])
                    for hs in halves:
                        nc.vector.tensor_tensor(
                            osb[h // 2][(h % 2) * 64:(h % 2) * 64 + 64, hs],
                            ps_o[0:64, hs], bc_sb[:, hs], op=mult)

                # --- filler units ---
                def outproj_full(ihalf, half, oc):
                    osb = osb_all[ihalf]
                    ps_out = ppC.tile([128, 512], f32, name="psC")
                    for kt in range(2):
                        nc.tensor.matmul(
                            ps_out[:, :],
                            lhsT=wo_sb[:, kt, oc * 128:(oc + 1) * 128],
                            rhs=osb[kt][:, half * 512:(half + 1) * 512],
                            start=(kt == 0), stop=(kt == 1),
                        )
                    o_t = outp.tile([128, 512], f32, name="o_t")
                    nc.vector.tensor_scalar_add(o_t[:], ps_out[:, :],
                                                b_sb[:, oc:oc + 1])
                    ib = 2 * ihalf + half
                    nc.sync.dma_start(rs_ins[ib][oc * 128:(oc + 1) * 128, :], o_t[:])

                def outproj_stash(half, oc):
                    # heads 0,1 (kt0) + head 2 (kt1 lower 64) + bias -> SBUF
                    ps_out = ppC.tile([128, 512], f32, name="psC")
                    nc.tensor.matmul(
                        ps_out[:, :],
                        lhsT=wo_sb[:, 0, oc * 128:(oc + 1) * 128],
                        rhs=osb_all[1][0][:, half * 512:(half + 1) * 512],
                        start=True, stop=False,
                    )
                    nc.tensor.matmul(
                        ps_out[:, :],
                        lhsT=wo_sb[0:64, 1, oc * 128:(oc + 1) * 128],
                        rhs=osb_all[1][1][0:64, half * 512:(half + 1) * 512],
                        start=False, stop=True,
                    )
                    with nc.allow_low_precision(reason="outproj partial stash"):
                        nc.vector.tensor_scalar_add(
                            o_part[:, O_SLOT[(half, oc)], :], ps_out[:, :],
                            b_sb[:, oc:oc + 1])

                def outproj_tail(half, oc):
                    eng = TAIL_ENG[(half, oc)]
                    if oc % 2 == 0:
                        ps_out = ppC.tile([128, 512], f32, name="psC")
                    else:
                        ps_out = ppS.tile([128, 512], f32, name="psA")
                    o_t = toutp.tile([128, 512], bf16, name="o_t2")
                    if eng == "act":
                        for kt in range(2):
                            nc.tensor.matmul(
                                ps_out[:, :],
                                lhsT=wo_sb[:, kt, oc * 128:(oc + 1) * 128],
                                rhs=osb_all[1][kt][:, half * 512:(half + 1) * 512],
                                start=(kt == 0), stop=(kt == 1),
                            )
                        with nc.allow_low_precision(reason="bf16 tail chunk"):
                            nc.scalar.activation(o_t[:], ps_out[:, :], Ident,
                                                 bias=b_sb[:, oc:oc + 1])
                    else:
                        nc.tensor.matmul(
                            ps_out[:, :],
                            lhsT=wo_sb[64:128, 1, oc * 128:(oc + 1) * 128],
                            rhs=osb_all[1][1][64:128, half * 512:(half + 1) * 512],
                            start=True, stop=True,
                        )
                        with nc.allow_low_precision(reason="bf16 tail chunk"):
                            nc.vector.tensor_tensor(
                                o_t[:], ps_out[:, :],
                                o_part[:, O_SLOT[(half, oc)], :], op=add)
                    dq = nc.sync if oc % 2 == 0 else nc.gpsimd
                    dq.dma_start(rs_ins[2 + half][oc * 128:(oc + 1) * 128, :], o_t[:])

                def deferred_qk_units(h, ic4):
                    isl = slice(ic4 * 512, (ic4 + 1) * 512)
                    xt = xts[ic4]
                    st = {}

                    def mm_pair(i):
                        def f():
                            if i == 0:
                                st["ps"] = ppC.tile([128, 512], f32, name="psC")
                            for kt in (2 * i, 2 * i + 1):
                                nc.tensor.matmul(
                                    st["ps"][:, :],
                                    lhsT=wqk[:, kt, h * 128:(h + 1) * 128],
                                    rhs=xt[:, kt, :],
                                    start=(kt == 0), stop=(kt == 7),
                                )
                        return f

                    def rope_unit():
                        qks = pscr.tile([128, 512], f32r, name="qks")
                        nc.vector.tensor_copy(qks[:], st["ps"][:, :])
                        rope_rot(h, isl, qks, pscr, ppC)()

                    return [mm_pair(i) for i in range(4)] + [rope_unit]

                def run_block(ihalf, h, fillers, nsl=2):
                    ps_o = ppO.tile([128, 1024], f32, name="psO")
                    fin = attn_seq(ihalf, h, ps_o, range(16), fillers)
                    while fillers:
                        fillers.pop(0)()
                    fin()
                    attn_norm(ihalf, h, ps_o, nsl)

                def rs_fire(ib):
                    dt = f32 if ib < 2 else bf16
                    dst = y_out[ib] if ib < 2 else y2_out[ib - 2]
                    if with_collective:
                        rs_out = dram.tile([256, 512], dt, name=f"rs_out_{ib}")
                        nc.gpsimd.collective_compute(
                            "ReduceScatter",
                            mybir.AluOpType.add,
                            replica_groups=GROUPS,
                            ins=[rs_ins[ib][:]],
                            outs=[rs_out[:]],
                        )
                        nc.sync.dma_start(dst, rs_out[:])
                    else:
                        nc.sync.dma_start(dst, rs_ins[ib][0:256, :])

                # finish interleaved block (0,0)
                ph1_fin[0]()
                attn_norm(0, 0, ps_o0)

                run_block(0, 1, deferred_qk_units(2, 2) + deferred_qk_units(2, 3))
                run_block(0, 2, deferred_qk_units(3, 2))
                run_block(0, 3, deferred_qk_units(3, 3))
                run_block(1, 0, [lambda oc=oc: outproj_full(0, 0, oc) for oc in range(8)])
                rs_fire(0)
                run_block(1, 1, [lambda oc=oc: outproj_full(0, 1, oc) for oc in range(5)])
                run_block(1, 2, [lambda oc=oc: outproj_full(0, 1, oc) for oc in range(5, 8)])
                rs_fire(1)
                run_block(1, 3, [lambda u=u: outproj_stash(*u)
                                 for u in sorted(O_SLOT, key=O_SLOT.get)], nsl=4)
                # tail: only head-3 matmuls (dve/pool units) or full 2-kt
                # (act units); ocs 0,1 first so the y copy can start early
                for half in range(2):
                    for oc in range(8):
                        outproj_tail(half, oc)
                    rs_fire(2 + half)

    nc.compile()
    return nc


def _get_nc():
    if "nc" not in _COMPILED:
        _COMPILED["nc"] = build_nc()
    return _COMPILED["nc"]


def kernel(x, w_qkv, w_out, b_out):
    from concourse import bass_utils

    x = np.asarray(x, dtype=np.float32)
    w_qkv = np.asarray(w_qkv, dtype=np.float32)
    w_out = np.asarray(w_out, dtype=np.float32)
    b_out = np.asarray(b_out, dtype=np.float32)

    nc = _get_nc()
    in_maps = _host_prep(x, w_qkv, w_out, b_out)
    res = bass_utils.run_bass_kernel_spmd(nc, in_maps, list(range(N_CORES)))

    out = np.zeros((B, N, DIM), np.float32)
    for c in range(N_CORES):
        g, pos = c // 4, c % 4
        y = res.results[c]["y"]  # [4, 256, 512] (ib 0,1 valid)
        y2 = np.asarray(res.results[c]["y2"]).astype(np.float32)
        for ib in range(4):
            blk = y[ib] if ib < 2 else y2[ib - 2]
            out[g, ib * 512:(ib + 1) * 512, pos * 256:(pos + 1) * 256] = blk.T
    return out


if __name__ == "__main__":
    rng = np.random.default_rng(0)
    x = rng.standard_normal((B, N, DIM)).astype(np.float32)
    w_qkv = (rng.standard_normal((DIM, 3 * DIM)) * DIM ** -0.5).astype(np.float32)
    w_out = (rng.standard_normal((DIM, DIM)) * DIM ** -0.5).astype(np.float32)
    b_out = np.zeros(DIM, np.float32)
    out = kernel(x, w_qkv, w_out, b_out)
    print("out", out.shape, out.dtype, float(np.abs(out).max()))


# revision 4
# speedup vs baseline: 1.0273x; 1.0082x over previous
"""Multi-head attention with RoPE on 8 Trainium2 NeuronCores — v2 schedule.

Same math/layout as v1 (core c -> batch g = c//4, head-group c%4; QKV via
column-sliced w_qkv; RoPE as signed-permutation matmul + elementwise; S^T =
K'Q'^T per 128-row j-tile; exp on ACT with no max-subtraction; ones-column
appended to V so the denominator accumulates in the same PSUM as P@V;
chunked ReduceScatter per 512-row i-block). v2 reworks the schedule around
the engine balance (PE ~167us, ACT-exp ~133us, DVE/Pool well under):

- softmax normalize fully off the PE: DVE reciprocal -> GpSimd
  partition_broadcast -> DVE multiply, emitted group-wise so the in-order
  DVE queue never head-blocks on Pool.
- attention inner loop emits with one-jt lookahead (scores jt+1 ahead of
  PV jt) plus dripped PE filler units, so the ACT-bound exp pipeline never
  starves the PE.
- h2/h3's qk projection for the second token half is deferred into the
  early phase-2 blocks as filler; scores get a dedicated PSUM pool so the
  projection/rope PSUM ring never gates them.
- out-projection tail: contributions of heads 0-2 (+bias) are stashed to
  SBUF during the last block; after the final norm only contraction-64
  matmuls for the last head plus a DVE/Pool/ACT-split finalize remain.
"""

import numpy as np
import ml_dtypes

H, HD = 16, 64
B, N, DIM = 2, 2048, 1024
N_CORES = 8
GROUPS = [[0, 1, 2, 3], [4, 5, 6, 7]]

_COMPILED = {}


def _host_prep(x, w_qkv, w_out, b_out):
    freqs = 10000.0 ** (-np.arange(0, HD, 2, dtype=np.float32) / HD)
    angles = np.arange(N, dtype=np.float32)[:, None] * freqs
    sin = np.sin(angles).astype(np.float32)
    cos = np.cos(angles).astype(np.float32)
    sin_i = np.stack([sin, sin], axis=-1).reshape(N, HD)
    cos_i = np.stack([cos, cos], axis=-1).reshape(N, HD)
    cs = np.concatenate([cos_i.T, cos_i.T], 0).copy()  # [128, N]
    sn = np.concatenate([sin_i.T, sin_i.T], 0).copy()

    R = np.zeros((HD, HD), np.float32)
    for d in range(32):
        R[d, 2 * d + 1] = -1.0
    for d in range(32, 64):
        R[d, 2 * (d - 32)] = 1.0
    R2 = np.zeros((128, 128), np.float32)
    R2[:64, :64] = R
    R2[64:, 64:] = R
    r2t = np.ascontiguousarray(R2.T)

    in_maps = []
    for c in range(N_CORES):
        g, hg = c // 4, c % 4
        heads = range(4 * hg, 4 * hg + 4)
        w_qk = np.concatenate(
            [np.concatenate([w_qkv[:, h * 64:(h + 1) * 64],
                             w_qkv[:, DIM + h * 64: DIM + (h + 1) * 64]], axis=1)
             for h in heads], axis=1)
        w_v = np.concatenate(
            [w_qkv[:, 2 * DIM + h * 64: 2 * DIM + (h + 1) * 64] for h in heads], axis=1)
        w_o = np.ascontiguousarray(w_out[4 * hg * 64:(4 * hg + 4) * 64, :])
        b_o = np.ascontiguousarray((b_out / 4.0).reshape(8, 128).T)
        in_maps.append({
            "x_t": np.ascontiguousarray(x[g].T).astype(ml_dtypes.bfloat16),
            "w_qk": np.ascontiguousarray(w_qk).astype(ml_dtypes.bfloat16),
            "w_v": np.ascontiguousarray(w_v).astype(ml_dtypes.bfloat16),
            "w_o": w_o,
            "b_o": b_o,
            "cs": cs,
            "sn": sn,
            "r2t": r2t,
            "ones": np.ones((128, 64), np.float32),
        })
    return in_maps


def build_nc(with_collective=True):
    import concourse.bass as bass  # noqa: F401
    import concourse.mybir as mybir
    import concourse.tile as tile
    from concourse import bacc

    f32 = mybir.dt.float32
    f32r = mybir.dt.float32r
    bf16 = mybir.dt.bfloat16
    mult = mybir.AluOpType.mult
    add = mybir.AluOpType.add
    Exp = mybir.ActivationFunctionType.Exp
    Ident = mybir.ActivationFunctionType.Identity

    nc = bacc.Bacc("TRN2", target_bir_lowering=False, debug=False,
                   num_devices=N_CORES)
    x_t = nc.dram_tensor("x_t", [DIM, N], bf16, kind="ExternalInput")
    w_qk = nc.dram_tensor("w_qk", [DIM, 512], bf16, kind="ExternalInput")
    w_v = nc.dram_tensor("w_v", [DIM, 256], bf16, kind="ExternalInput")
    w_o = nc.dram_tensor("w_o", [256, DIM], f32r, kind="ExternalInput")
    b_o = nc.dram_tensor("b_o", [128, 8], f32, kind="ExternalInput")
    cs_d = nc.dram_tensor("cs", [128, N], f32, kind="ExternalInput")
    sn_d = nc.dram_tensor("sn", [128, N], f32, kind="ExternalInput")
    r2t_d = nc.dram_tensor("r2t", [128, 128], f32r, kind="ExternalInput")
    ones_d = nc.dram_tensor("ones", [128, 64], f32r, kind="ExternalInput")
    y_out = nc.dram_tensor("y", [4, 256, 512], f32, kind="ExternalOutput")
    y2_out = nc.dram_tensor("y2", [2, 256, 512], bf16, kind="ExternalOutput")

    # tail finalize engine per (half, oc); dve/pool units use a stash slot
    TAIL_ENG = {}
    for half in range(2):
        for oc in range(8):
            TAIL_ENG[(half, oc)] = "act" if oc % 2 == 1 else "dve"
    O_SLOT = {u: i for i, u in enumerate(
        u for u in TAIL_ENG if TAIL_ENG[u] != "act")}

    with tile.TileContext(nc) as tc:
        with (
            tc.tile_pool(name="persist", bufs=1) as persist,
            tc.tile_pool(name="xtp", bufs=4) as xtp,
            tc.tile_pool(name="ppS", bufs=2, space="PSUM") as ppS,
            tc.tile_pool(name="ppO", bufs=1, space="PSUM") as ppO,
            tc.tile_pool(name="ppC", bufs=2, space="PSUM") as ppC,
            tc.tile_pool(name="dram", bufs=8, space="DRAM") as dram,
            tc.tile_pool(name="epool", bufs=8) as epool,
        ):
            qp = persist.tile([64, 4, N], f32r)            # q'^T per head [d64, n]
            kp = persist.tile([64, 4, N], f32r)            # k'^T per head [d64, n]
            vsb = persist.tile([128, 16, 4, 65], bf16)     # v + ones col, per j-tile
            wo_sb = persist.tile([128, 2, DIM], f32r)
            b_sb = persist.tile([128, 8], f32)
            wqk = persist.tile([128, 8, 512], bf16)
            cs_hi = persist.tile([128, 1024], f32)
            sn_hi = persist.tile([128, 1024], f32)
            r2t_sb = persist.tile([128, 128], f32r)

            def cs_at(isl):  # cos slice [128, 512] for token slice isl
                return (cs_hi[:, isl.start - 1024:isl.stop - 1024]
                        if isl.start >= 1024 else cs_lo[:, isl])

            def sn_at(isl):
                return (sn_hi[:, isl.start - 1024:isl.stop - 1024]
                        if isl.start >= 1024 else sn_lo[:, isl])

            def qk_mms(h, xt, pool):
                ps_qk = pool.tile([128, 512], f32, name="psC")
                for kt in range(8):
                    nc.tensor.matmul(
                        ps_qk[:, :],
                        lhsT=wqk[:, kt, h * 128:(h + 1) * 128],
                        rhs=xt[:, kt, :],
                        start=(kt == 0), stop=(kt == 7),
                    )
                return ps_qk

            def qk_copy(h, ps_qk, scrp):
                # alternate ACT/DVE so ring slots free independently
                qks = scrp.tile([128, 512], f32r, name="qks")
                if h % 2 == 0:
                    nc.scalar.copy(qks[:], ps_qk[:, :])
                else:
                    nc.vector.tensor_copy(qks[:], ps_qk[:, :])
                return qks

            # rope rotation + t1/t2; qp/kp adds returned as deferred closure
            def rope_rot(h, isl, qks, scrp, pool):
                ps_rot = pool.tile([128, 512], f32, name="psC")
                nc.tensor.matmul(ps_rot[:, :], lhsT=r2t_sb[:],
                                 rhs=qks[:], start=True, stop=True)
                t1 = scrp.tile([128, 512], f32, name="t1")
                nc.gpsimd.tensor_tensor(t1[:], qks[:].bitcast(f32), cs_at(isl), op=mult)
                t2 = scrp.tile([128, 512], f32, name="t2")
                nc.vector.tensor_tensor(t2[:], ps_rot[:, :], sn_at(isl), op=mult)

                def adds():
                    nc.gpsimd.tensor_tensor(kp[:, h, isl], t1[64:128, :], t2[64:128, :], op=add)
                    nc.vector.tensor_tensor(qp[:, h, isl], t1[0:64, :], t2[0:64, :], op=add)
                return adds

            # ---- attention emitter with one-jt lookahead + PE fillers ----
            def attn_seq(ihalf, h, ps_o, jts, fillers):
                pend_pv = [None]

                def emit_pv(jt, e_t):
                    for half in range(2):
                        nc.tensor.matmul(
                            ps_o[0:65, half * 512:(half + 1) * 512],
                            lhsT=vsb[:, jt, h, :],
                            rhs=e_t[:, half * 512:(half + 1) * 512],
                            start=(jt == 0), stop=(jt == 15),
                        )

                for jt in jts:
                    ps_s = ppS.tile([128, 1024], f32, name="psA")
                    for half in range(2):
                        nc.tensor.matmul(
                            ps_s[:, half * 512:(half + 1) * 512],
                            lhsT=kp[:, h, jt * 128:(jt + 1) * 128],
                            rhs=qp[:, h,
                                   ihalf * 1024 + half * 512:
                                   ihalf * 1024 + (half + 1) * 512],
                            start=True, stop=True,
                        )
                    e_t = epool.tile([128, 1024], bf16, name="e_t")
                    nc.scalar.activation(e_t[:], ps_s[:], Exp, scale=0.125)
                    if pend_pv[0] is not None:
                        if fillers:
                            fillers.pop(0)()
                        emit_pv(*pend_pv[0])
                    pend_pv[0] = (jt, e_t)

                def finish():
                    emit_pv(*pend_pv[0])
                return finish

            # ---------------- Phase 1 ----------------
            with (
                tc.tile_pool(name="xw", bufs=1) as xw,
                tc.tile_pool(name="scr", bufs=3) as scr,
            ):
                cs_lo = xw.tile([128, 1024], f32)
                sn_lo = xw.tile([128, 1024], f32)
                wv = xw.tile([128, 8, 256], bf16)
                xt0 = xtp.tile([128, 8, 512], bf16, name="xt")
                # keep the gpsimd/SWDGE queue nearly empty in phase 1: its
                # descriptor processing occupies the Pool ENGINE (~1us per
                # transfer), which phase 1 needs for rope t1/kp-adds.
                for kt in range(4):
                    nc.sync.dma_start(wqk[:, kt, :], w_qk[kt * 128:(kt + 1) * 128, :])
                    nc.sync.dma_start(xt0[:, kt, :], x_t[kt * 128:(kt + 1) * 128, 0:512])
                for kt in range(4, 8):
                    nc.gpsimd.dma_start(wqk[:, kt, :], w_qk[kt * 128:(kt + 1) * 128, :])
                for kt in range(4, 8):
                    nc.sync.dma_start(xt0[:, kt, :], x_t[kt * 128:(kt + 1) * 128, 0:512])
                nc.sync.dma_start(r2t_sb[:], r2t_d.ap())
                nc.sync.dma_start(cs_lo[:], cs_d[:, 0:1024])
                nc.sync.dma_start(sn_lo[:], sn_d[:, 0:1024])
                for kt in range(8):
                    nc.sync.dma_start(wv[:, kt, :], w_v[kt * 128:(kt + 1) * 128, :])
                ones_stage = xw.tile([128, 64], f32r)
                nc.gpsimd.dma_start(ones_stage[:], ones_d[:, :])
                nc.scalar.copy(
                    vsb[:, :, :, 64:65],
                    ones_stage[:, :].rearrange("p (a b c) -> p a b c", b=4, c=1))
                nc.sync.dma_start(cs_hi[:], cs_d[:, 1024:2048])
                nc.sync.dma_start(sn_hi[:], sn_d[:, 1024:2048])
                nc.gpsimd.dma_start(b_sb[:], b_o.ap())
                for kt in range(2):
                    nc.sync.dma_start(wo_sb[:, kt, :], w_o[kt * 128:(kt + 1) * 128, :])

                xts = {0: xt0}
                ph1_fin = [None]
                JTS_TOP = {2: range(4, 8), 3: range(10, 12)}
                JTS_END = {1: range(0, 4), 2: range(8, 10), 3: range(12, 16)}
                for ic4 in range(4):
                    isl = slice(ic4 * 512, (ic4 + 1) * 512)
                    if ic4 < 3:
                        nsl = slice((ic4 + 1) * 512, (ic4 + 2) * 512)
                        xn = xtp.tile([128, 8, 512], bf16, name="xt")
                        xts[ic4 + 1] = xn
                        for kt in range(8):
                            nc.sync.dma_start(xn[:, kt, :], x_t[kt * 128:(kt + 1) * 128, nsl])
                    xt = xts[ic4]
                    if ic4 in JTS_TOP:
                        ph1_fin[0]()
                        ph1_fin[0] = attn_seq(0, 0, ps_o0, JTS_TOP[ic4], [])
                    heads = list(range(4) if ic4 < 2 else range(2))
                    defer = 2 if len(heads) == 4 else 1
                    qks_of = {}
                    adds = []
                    n_rot = 0

                    def emit_rot(hh):
                        adds.append(rope_rot(hh, isl, qks_of[hh], scr, ppC))
                        if len(adds) > 1:
                            adds.pop(0)()

                    for idx, h in enumerate(heads):
                        ps_qk = qk_mms(h, xt, ppC)
                        qks_of[h] = qk_copy(h, ps_qk, scr)
                        while idx - n_rot >= defer:
                            emit_rot(heads[n_rot])
                            n_rot += 1
                    while n_rot < len(heads):
                        emit_rot(heads[n_rot])
                        n_rot += 1
                    for it2 in range(4):
                        it = ic4 * 4 + it2
                        ps_v = ppC.tile([128, 512], f32, name="psC")
                        for kt in range(8):
                            nc.tensor.matmul(
                                ps_v[:, 0:256],
                                lhsT=xt[:, kt, it2 * 128:(it2 + 1) * 128],
                                rhs=wv[:, kt, :],
                                start=(kt == 0), stop=(kt == 7),
                            )
                        nc.vector.tensor_copy(
                            vsb[:, it, :, 0:64],
                            ps_v[:, 0:256].rearrange("p (h d) -> p h d", d=64),
                        )
                    while adds:
                        adds.pop(0)()
                    if ic4 == 1:
                        ps_o0 = ppO.tile([128, 1024], f32, name="psO")
                    if ic4 >= 1:
                        if ph1_fin[0] is not None:
                            ph1_fin[0]()
                        ph1_fin[0] = attn_seq(0, 0, ps_o0, JTS_END[ic4], [])

            # ---------------- Phase 2 ----------------
            with (
                tc.tile_pool(name="opool", bufs=1) as opool,
                tc.tile_pool(name="npool", bufs=2) as npool,
                tc.tile_pool(name="outp", bufs=4) as outp,
                tc.tile_pool(name="toutp", bufs=8) as toutp,
                tc.tile_pool(name="pscr", bufs=1) as pscr,
            ):
                osb_all = {
                    0: [opool.tile([128, 1024], f32r, name=f"osb0_{kt}") for kt in range(2)],
                    1: [opool.tile([128, 1024], f32r, name=f"osb1_{kt}") for kt in range(2)],
                }
                # heads 0-2 + bias partials of the last i-half's out-proj
                o_part = opool.tile([128, len(O_SLOT), 512], bf16)
                rs_ins = {ib: dram.tile([1024, 512], f32 if ib < 2 else bf16,
                                         name=f"rs_in_{ib}")
                          for ib in range(4)}

                def attn_norm(ihalf, h, ps_o, nsl=2):
                    # grouped emission: recips, then broadcasts, then mults
                    osb = osb_all[ihalf]
                    recip = npool.tile([1, 1024], f32r, name="recip")
                    bc_sb = npool.tile([64, 1024], f32r, name="bc_sb")
                    w = 1024 // nsl
                    halves = [slice(i * w, (i + 1) * w) for i in range(nsl)]
                    for hs in halves:
                        with nc.allow_low_precision(reason="softmax denom recip"):
                            nc.vector.reciprocal(recip[:, hs], ps_o[64:65, hs])
                    for hs in halves:
                        nc.gpsimd.partition_broadcast(bc_sb[:, hs], recip[0:1, hs])
                    for hs in halves:
                        nc.vector.tensor_tensor(
                            osb[h // 2][(h % 2) * 64:(h % 2) * 64 + 64, hs],
                            ps_o[0:64, hs], bc_sb[:, hs], op=mult)

                # --- filler units ---
                def outproj_full(ihalf, half, oc):
                    osb = osb_all[ihalf]
                    ps_out = ppC.tile([128, 512], f32, name="psC")
                    for kt in range(2):
                        nc.tensor.matmul(
                            ps_out[:, :],
                            lhsT=wo_sb[:, kt, oc * 128:(oc + 1) * 128],
                            rhs=osb[kt][:, half * 512:(half + 1) * 512],
                            start=(kt == 0), stop=(kt == 1),
                        )
                    o_t = outp.tile([128, 512], f32, name="o_t")
                    nc.vector.tensor_scalar_add(o_t[:], ps_out[:, :],
                                                b_sb[:, oc:oc + 1])
                    ib = 2 * ihalf + half
                    nc.sync.dma_start(rs_ins[ib][oc * 128:(oc + 1) * 128, :], o_t[:])

                def outproj_stash(half, oc):
                    # heads 0,1 (kt0) + head 2 (kt1 lower 64) + bias -> SBUF
                    ps_out = ppC.tile([128, 512], f32, name="psC")
                    nc.tensor.matmul(
                        ps_out[:, :],
                        lhsT=wo_sb[:, 0, oc * 128:(oc + 1) * 128],
                        rhs=osb_all[1][0][:, half * 512:(half + 1) * 512],
                        start=True, stop=False,
                    )
                    nc.tensor.matmul(
                        ps_out[:, :],
                        lhsT=wo_sb[0:64, 1, oc * 128:(oc + 1) * 128],
                        rhs=osb_all[1][1][0:64, half * 512:(half + 1) * 512],
                        start=False, stop=True,
                    )
                    with nc.allow_low_precision(reason="outproj partial stash"):
                        nc.vector.tensor_scalar_add(
                            o_part[:, O_SLOT[(half, oc)], :], ps_out[:, :],
                            b_sb[:, oc:oc + 1])

                def outproj_tail(half, oc):
                    eng = TAIL_ENG[(half, oc)]
                    if oc % 2 == 0:
                        ps_out = ppC.tile([128, 512], f32, name="psC")
                    else:
                        ps_out = ppS.tile([128, 512], f32, name="psA")
                    o_t = toutp.tile([128, 512], bf16, name="o_t2")
                    if eng == "act":
                        for kt in range(2):
                            nc.tensor.matmul(
                                ps_out[:, :],
                                lhsT=wo_sb[:, kt, oc * 128:(oc + 1) * 128],
                                rhs=osb_all[1][kt][:, half * 512:(half + 1) * 512],
                                start=(kt == 0), stop=(kt == 1),
                            )
                        with nc.allow_low_precision(reason="bf16 tail chunk"):
                            nc.scalar.activation(o_t[:], ps_out[:, :], Ident,
                                                 bias=b_sb[:, oc:oc + 1])
                    else:
                        nc.tensor.matmul(
                            ps_out[:, :],
                            lhsT=wo_sb[64:128, 1, oc * 128:(oc + 1) * 128],
                            rhs=osb_all[1][1][64:128, half * 512:(half + 1) * 512],
                            start=True, stop=True,
                        )
                        with nc.allow_low_precision(reason="bf16 tail chunk"):
                            nc.vector.tensor_tensor(
                                o_t[:], ps_out[:, :],
                                o_part[:, O_SLOT[(half, oc)], :], op=add)
                    dq = nc.sync if oc % 2 == 0 else nc.gpsimd
                    dq.dma_start(rs_ins[2 + half][oc * 128:(oc + 1) * 128, :], o_t[:])

                def deferred_qk_units(h, ic4):
                    isl = slice(ic4 * 512, (ic4 + 1) * 512)
                    xt = xts[ic4]
                    st = {}

                    def mm_pair(i):
                        def f():
                            if i == 0:
                                st["ps"] = ppC.tile([128, 512], f32, name="psC")
                            for kt in (2 * i, 2 * i + 1):
                                nc.tensor.matmul(
                                    st["ps"][:, :],
                                    lhsT=wqk[:, kt, h * 128:(h + 1) * 128],
                                    rhs=xt[:, kt, :],
                                    start=(kt == 0), stop=(kt == 7),
                                )
                        return f

                    def rope_unit():
                        qks = pscr.tile([128, 512], f32r, name="qks")
                        nc.vector.tensor_copy(qks[:], st["ps"][:, :])
                        rope_rot(h, isl, qks, pscr, ppC)()

                    return [mm_pair(i) for i in range(4)] + [rope_unit]

                def run_block(ihalf, h, fillers, nsl=2):
                    ps_o = ppO.tile([128, 1024], f32, name="psO")
                    fin = attn_seq(ihalf, h, ps_o, range(16), fillers)
                    while fillers:
                        fillers.pop(0)()
                    fin()
                    attn_norm(ihalf, h, ps_o, nsl)

                def rs_fire(ib):
                    dt = f32 if ib < 2 else bf16
                    dst = y_out[ib] if ib < 2 else y2_out[ib - 2]
                    if with_collective:
                        rs_out = dram.tile([256, 512], dt, name=f"rs_out_{ib}")
                        nc.gpsimd.collective_compute(
                            "ReduceScatter",
                            mybir.AluOpType.add,
                            replica_groups=GROUPS,
                            ins=[rs_ins[ib][:]],
                            outs=[rs_out[:]],
                        )
                        nc.sync.dma_start(dst, rs_out[:])
                    else:
                        nc.sync.dma_start(dst, rs_ins[ib][0:256, :])

                # finish interleaved block (0,0)
                ph1_fin[0]()
                attn_norm(0, 0, ps_o0)

                run_block(0, 1, deferred_qk_units(2, 2) + deferred_qk_units(2, 3))
                run_block(0, 2, deferred_qk_units(3, 2))
                run_block(0, 3, deferred_qk_units(3, 3))
                run_block(1, 0, [lambda oc=oc: outproj_full(0, 0, oc) for oc in range(8)])
                rs_fire(0)
                run_block(1, 1, [lambda oc=oc: outproj_full(0, 1, oc) for oc in range(5)])
                run_block(1, 2, [lambda oc=oc: outproj_full(0, 1, oc) for oc in range(5, 8)])
                rs_fire(1)
                run_block(1, 3, [lambda u=u: outproj_stash(*u)
                                 for u in sorted(O_SLOT, key=O_SLOT.get)], nsl=4)
                # tail: only head-3 matmuls (dve/pool units) or full 2-kt
                # (act units); ocs 0,1 first so the y copy can start early
                for half in range(2):
                    for oc in range(8):
                        outproj_tail(half, oc)
                    rs_fire(2 + half)

    nc.compile()
    return nc


def _get_nc():
    if "nc" not in _COMPILED:
        _COMPILED["nc"] = build_nc()
    return _COMPILED["nc"]


def kernel(x, w_qkv, w_out, b_out):
    from concourse import bass_utils

    x = np.asarray(x, dtype=np.float32)
    w_qkv = np.asarray(w_qkv, dtype=np.float32)
    w_out = np.asarray(w_out, dtype=np.float32)
    b_out = np.asarray(b_out, dtype=np.float32)

    nc = _get_nc()
    in_maps = _host_prep(x, w_qkv, w_out, b_out)
    res = bass_utils.run_bass_kernel_spmd(nc, in_maps, list(range(N_CORES)))

    out = np.zeros((B, N, DIM), np.float32)
    for c in range(N_CORES):
        g, pos = c // 4, c % 4
        y = res.results[c]["y"]  # [4, 256, 512] (ib 0,1 valid)
        y2 = np.asarray(res.results[c]["y2"]).astype(np.float32)
        for ib in range(4):
            blk = y[ib] if ib < 2 else y2[ib - 2]
            out[g, ib * 512:(ib + 1) * 512, pos * 256:(pos + 1) * 256] = blk.T
    return out


if __name__ == "__main__":
    rng = np.random.default_rng(0)
    x = rng.standard_normal((B, N, DIM)).astype(np.float32)
    w_qkv = (rng.standard_normal((DIM, 3 * DIM)) * DIM ** -0.5).astype(np.float32)
    w_out = (rng.standard_normal((DIM, DIM)) * DIM ** -0.5).astype(np.float32)
    b_out = np.zeros(DIM, np.float32)
    out = kernel(x, w_qkv, w_out, b_out)
    print("out", out.shape, out.dtype, float(np.abs(out).max()))
